# revision 3
# baseline (speedup 1.0000x reference)
"""Trainium2 Bass kernel for DynamicCondLinear (MoE-routing style).

Math: condition batch is 1, so the softmax routing weights (K=8) are shared by
all 32 samples; out = sum_k a_k * (x @ W_k^T) + sum_k a_k * b_k with
a = softmax(relu(cond @ w1 + b1) @ w2 + b2).

Sharding: tensor-parallel over OUT channels (2048 / 8 cores = 256 per core).
Each core streams its 8 MiB fp16 weight shard from HBM once; that stream is
the roofline (~20 us at the 412 GB/s measured 8KiB-packet rate).

v2 schedule (trace-driven redesign of the 43.3 us baseline):
 - per-k PSUM groups: slab k's matmuls accumulate raw x @ W_k^T into a
   dedicated (B, OC) PSUM slice, so the main contraction starts as soon as
   x + slab 0 land (~4 us) and never waits on the alpha MLP. The e-weighted
   combine runs as one fused DVE op per k (scalar_tensor_tensor:
   acc' = psum_k * e_k + acc), staggered with slab completions; only the
   k=7 op is on the tail.
 - dual HWDGE queues: slabs alternate between the sync and scalar rings so
   descriptor generation and per-DMA issue overheads overlap; all big
   transfers use >=2 KiB descriptors. Last slab lands in 4 column quarters
   to keep the PE trailing the final bytes by <0.5 us.
 - w1 ships as fp8e4m3 (1 MiB instead of 2 MiB bf16): measured end-to-end
   rel err 6.9e-3 vs the 2e-2 gate. The PE upconverts fp8 rhs against the
   fp16 condition stationary.
 - softmax normalization is folded into the routing weights (e_k / sum(e))
   before the combine, so the output needs no final rescale pass.
 - dependency-free warm-up matmuls hold the PE HAM clock up through the DMA
   prefix; a tail batch keeps the sequencer hot into the NEFF epilogue's
   distributed semaphore clear (which otherwise runs at the LOW p-state).

Host-side prep is layout-only (transpose/reshape/cast for DMA-friendly
tiling); all math happens on-device.
"""

import os
import sys

import numpy as np

if "/opt/trn_rl_repo" not in sys.path:
    sys.path.insert(0, "/opt/trn_rl_repo")

import concourse.bacc as bacc
import concourse.mybir as mybir
import concourse.tile as tile
from concourse.bass_utils import run_bass_kernel_spmd

B, IN, OUT, K, H = 32, 2048, 2048, 8, 512
NCORES = 8
OC = OUT // NCORES  # 256 out channels per core
JT = IN // 128      # 16 contraction tiles
HT = H // 128       # 4 hidden tiles

F32 = mybir.dt.float32
FP16 = mybir.dt.float16
FP8 = mybir.dt.float8e4

CXW = JT + JT * B + HT * K   # fp16 pack: ct | xt | w2t  (560 cols)
XOFF = JT
WOFF = JT + JT * B

_CACHE = {}
LAST_RESULTS = None  # test.py reads this for profiling info


def _build_module():
    nc = bacc.Bacc("TRN2", target_bir_lowering=False, debug=False,
                   num_devices=NCORES)

    wt_d = nc.dram_tensor("wt", (K, 128, JT * OC), FP16, kind="ExternalInput")
    cxw_d = nc.dram_tensor("cxw", (128, CXW), FP16, kind="ExternalInput")
    w18_d = nc.dram_tensor("w18", (128, JT * H), FP8, kind="ExternalInput")
    b1r_d = nc.dram_tensor("b1r", (1, H), FP16, kind="ExternalInput")
    b2r_d = nc.dram_tensor("b2r", (1, K), FP16, kind="ExternalInput")
    kb_d = nc.dram_tensor("kb", (K, OC), FP16, kind="ExternalInput")
    y_d = nc.dram_tensor("y", (B, OC), F32, kind="ExternalOutput")
    # warmup sink: consumed so bacc's DCE keeps the PE warm-up matmuls
    ysink_d = nc.dram_tensor("ysink", (1, 1), F32, kind="ExternalOutput")

    n_warm1 = int(os.environ.get("KERNEL_WARMUP1", "12"))
    n_tailwarm = int(os.environ.get("KERNEL_TAILWARM", "28"))

    with tile.TileContext(nc) as tc:
        with (
            tc.tile_pool(name="cpool", bufs=1) as cpool,
            tc.tile_pool(name="wpool", bufs=1) as wpool,
            tc.tile_pool(name="ppool", bufs=1, space="PSUM") as ppool,
        ):
            # --- DMA: slabs alternate queues; MLP inputs lead the scalar
            # ring so the alpha path starts early. Slab 7 lands in 4
            # column quarters (2 per queue) for a tight tail. ---
            slabs = [wpool.tile((128, JT * OC), FP16, tag="wt_slab", bufs=K,
                                name=f"wt_slab{k}")
                     for k in range(K)]
            cxw_sb = cpool.tile((128, CXW), FP16)
            w18_sb = cpool.tile((128, JT * H), FP8)
            kb_sb = cpool.tile((K, OC), FP16)
            b1r_sb = cpool.tile((1, H), FP16)
            b2r_sb = cpool.tile((1, K), FP16)

            # sync ring: slabs 0,2,4,6 + quarters a,c of slab 7
            for k in (0, 2, 4, 6):
                nc.sync.dma_start(slabs[k][:], wt_d.ap()[k])
            QC = JT * OC // 4  # 1024 cols per quarter
            for q in (0, 2):
                nc.sync.dma_start(slabs[7][:, q * QC:(q + 1) * QC],
                                  wt_d.ap()[7][:, q * QC:(q + 1) * QC])

            # scalar ring: cxw pack, small biases, w1-fp8 chunks, slabs
            # 1,3,5 + quarters b,d of slab 7
            nc.scalar.dma_start(cxw_sb[:], cxw_d.ap())
            nc.scalar.dma_start(kb_sb[:], kb_d.ap())
            nc.scalar.dma_start(b1r_sb[:], b1r_d.ap())
            nc.scalar.dma_start(b2r_sb[:], b2r_d.ap())
            WC = JT * H // 4  # 2048 cols per w1 chunk
            for c in range(4):
                nc.scalar.dma_start(w18_sb[:, c * WC:(c + 1) * WC],
                                    w18_d.ap()[:, c * WC:(c + 1) * WC])
            for k in (1, 3, 5):
                nc.scalar.dma_start(slabs[k][:], wt_d.ap()[k])
            for q in (1, 3):
                nc.scalar.dma_start(slabs[7][:, q * QC:(q + 1) * QC],
                                    wt_d.ap()[7][:, q * QC:(q + 1) * QC])

            # --- constants ---
            one1h = cpool.tile((1, 1), FP16)
            nc.gpsimd.memset(one1h[:], 1.0)
            ones_b = cpool.tile((1, B), FP16)
            nc.gpsimd.memset(ones_b[:], 1.0)

            # --- PE warm-up: dependency-free matmuls ramp the HAM clock ---
            dum_a = cpool.tile((128, B), FP16)
            nc.gpsimd.memset(dum_a[:], 0.0)
            dum_b = cpool.tile((128, OC), FP16)
            nc.gpsimd.memset(dum_b[:], 0.0)
            dum_psum = ppool.tile((B, OC), F32)
            dum_sink = cpool.tile((1, 1), F32)

            def warmup(n):
                for _ in range(n):
                    nc.tensor.matmul(dum_psum[:], dum_a[:], dum_b[:],
                                     start=True, stop=True)

            warmup(n_warm1)

            # --- main contraction: one PSUM slice per k, raw x @ W_k^T.
            # PE program order interleaves slab groups with the MLP so
            # whichever data lands first keeps the PE busy. ---
            mpsum = ppool.tile((B, K * OC), F32)

            def slab_group(k):
                for j in range(JT):
                    nc.tensor.matmul(
                        mpsum[:, k * OC:(k + 1) * OC],
                        cxw_sb[:, XOFF + j * B:XOFF + (j + 1) * B],
                        slabs[k][:, j * OC:(j + 1) * OC],
                        start=(j == 0), stop=(j == JT - 1),
                    )

            slab_group(0)

            # --- alpha MLP: h = relu(cond @ w1 + b1), fp8 weights ---
            psum_h = ppool.tile((1, H), F32, tag="pA")
            for t in range(JT):
                nc.tensor.matmul(
                    psum_h[:],
                    cxw_sb[:, t:t + 1],                  # ct column t
                    w18_sb[:, t * H:(t + 1) * H],        # w1 fp8 slab t
                    start=(t == 0), stop=False,
                )
            nc.tensor.matmul(psum_h[:], one1h[:], b1r_sb[:],
                             start=False, stop=True)
            h_sb = cpool.tile((1, H), FP16)
            nc.scalar.activation(h_sb[:], psum_h[:],
                                 mybir.ActivationFunctionType.Relu)

            slab_group(1)

            # transpose h (1,512) -> hT (128,4) via tiny matmuls vs ones
            psum_ht = ppool.tile((128, HT), F32, tag="pB")
            for q in range(HT):
                nc.tensor.matmul(
                    psum_ht[:, q:q + 1],
                    h_sb[:, q * 128:(q + 1) * 128],
                    one1h[:],
                    start=True, stop=True,
                )
            ht_sb = cpool.tile((128, HT), FP16)
            nc.vector.tensor_copy(ht_sb[:], psum_ht[:])

            # scores row (1, 8) = sum_q hT[:,q].T @ w2t[:,q,:] + b2
            psum_s = ppool.tile((1, K), F32, tag="pC")
            for q in range(HT):
                nc.tensor.matmul(
                    psum_s[:],
                    ht_sb[:, q:q + 1],
                    cxw_sb[:, WOFF + q * K:WOFF + (q + 1) * K],
                    start=(q == 0), stop=False,
                )
            nc.tensor.matmul(psum_s[:], one1h[:], b2r_sb[:],
                             start=False, stop=True)

            slab_group(2)

            # softmax, normalized up front: en = exp(s) / sum(exp(s));
            # no max-subtraction (scores are O(1) for this model family)
            e_sb = cpool.tile((1, K), F32)
            nc.scalar.activation(e_sb[:], psum_s[:],
                                 mybir.ActivationFunctionType.Exp)
            esum = cpool.tile((1, 1), F32)
            nc.vector.reduce_sum(esum[:], e_sb[:], axis=mybir.AxisListType.X)
            rinv = cpool.tile((1, 1), F32)
            nc.vector.reciprocal(rinv[:], esum[:])
            en16 = cpool.tile((1, K), FP16)
            nc.vector.tensor_scalar_mul(en16[:], e_sb[:], rinv[:])

            # broadcast en to all B partitions (for the combine scalars)
            psum_e = ppool.tile((B, K), F32, tag="pB")
            nc.tensor.matmul(psum_e[:], ones_b[:], en16[:],
                             start=True, stop=True)
            e32_sb = cpool.tile((B, K), F32)
            nc.vector.tensor_copy(e32_sb[:], psum_e[:])

            # en column (8,1) -> e-weighted bias row (1, OC) -> broadcast
            # to (B, OC): the combine chain's initial accumulator
            psum_ac = ppool.tile((K, 1), F32, tag="pA")
            nc.tensor.matmul(psum_ac[:], en16[:], one1h[:],
                             start=True, stop=True)
            e_c = cpool.tile((K, 1), FP16)
            nc.vector.tensor_copy(e_c[:], psum_ac[:])
            psum_bb = ppool.tile((1, OC), F32, tag="pC")
            nc.tensor.matmul(psum_bb[:], e_c[:], kb_sb[:],
                             start=True, stop=True)
            aggb_sb = cpool.tile((1, OC), FP16)
            nc.vector.tensor_copy(aggb_sb[:], psum_bb[:])
            psum_cb = ppool.tile((B, OC), F32, tag="pA")
            nc.tensor.matmul(psum_cb[:], ones_b[:], aggb_sb[:],
                             start=True, stop=True)

            for k in range(3, K):
                slab_group(k)

            # --- tail warm: keep the PE sequencer at speed through the
            # NEFF epilogue's distributed semaphore clear ---
            warmup(n_tailwarm)
            nc.vector.tensor_copy(dum_sink[:], dum_psum[0:1, 0:1])
            nc.scalar.dma_start(ysink_d.ap(), dum_sink[:])

            # --- combine: acc' = psum_k * en_k + acc, one fused DVE op
            # per k; k<7 complete during the stream, only k=7 is tail ---
            acc_a = cpool.tile((B, OC), F32)
            acc_b = cpool.tile((B, OC), F32)
            y_sb = cpool.tile((B, OC), F32)
            nc.vector.tensor_copy(acc_a[:], psum_cb[:])
            cur, nxt = acc_a, acc_b
            for k in range(K):
                dst = y_sb if k == K - 1 else nxt
                nc.vector.scalar_tensor_tensor(
                    dst[:],
                    mpsum[:, k * OC:(k + 1) * OC],
                    e32_sb[:, k:k + 1],
                    cur[:],
                    op0=mybir.AluOpType.mult,
                    op1=mybir.AluOpType.add,
                )
                cur, nxt = dst, cur

            nc.scalar.dma_start(y_d.ap(), y_sb[:])

    nc.compile()
    return nc


def _prep_inputs(x, condition, w1, b1, w2, b2, kernels_weights, kernels_bias):
    """Layout-only host prep: slice per-core shards and retile for DMA."""
    import ml_dtypes
    f = np.float32
    f16 = np.float16
    f8 = ml_dtypes.float8_e4m3
    x = np.asarray(x, f)
    condition = np.asarray(condition, f)
    w1 = np.asarray(w1, f)
    b1 = np.asarray(b1, f)
    w2 = np.asarray(w2, f)
    b2 = np.asarray(b2, f)
    kernels_weights = np.asarray(kernels_weights, f)
    kernels_bias = np.asarray(kernels_bias, f)

    # xT tiled: xt[p, j*B + b] = x[b, j*128 + p]
    xt = np.ascontiguousarray(
        x.T.reshape(JT, 128, B).transpose(1, 0, 2)).reshape(128, JT * B)
    # w2 tiled as rhs: w2t[p, q*K + k] = w2[q*128 + p, k]
    w2t = np.ascontiguousarray(
        w2.reshape(HT, 128, K).transpose(1, 0, 2)).reshape(128, HT * K)
    ct = np.ascontiguousarray(condition.reshape(JT, 128).T)  # (128, JT)
    cxw = np.concatenate([ct, xt, w2t], axis=1).astype(f16)
    cxw = np.ascontiguousarray(cxw)

    # w1 tiled fp8: w18[p, t*H + h] = w1[t*128 + p, h]
    w18 = np.ascontiguousarray(
        w1.reshape(JT, 128, H).transpose(1, 0, 2)).reshape(128, JT * H)
    w18 = w18.astype(f8)

    b1r = np.ascontiguousarray(b1.reshape(1, H)).astype(f16)
    b2r = np.ascontiguousarray(b2.reshape(1, K)).astype(f16)

    in_maps = []
    for c in range(NCORES):
        osl = slice(c * OC, (c + 1) * OC)
        # W shard [k, o, i] -> tiles [k, p, j, o] with i = j*128 + p
        wt = np.ascontiguousarray(
            kernels_weights[:, osl, :].reshape(K, OC, JT, 128)
            .transpose(0, 3, 2, 1)).reshape(K, 128, JT * OC).astype(f16)
        kb = np.ascontiguousarray(kernels_bias[:, osl]).astype(f16)
        in_maps.append({
            "wt": wt, "cxw": cxw, "w18": w18,
            "b1r": b1r, "b2r": b2r, "kb": kb,
        })
    return in_maps


def kernel(x, condition, w1, b1, w2, b2, kernels_weights, kernels_bias):
    global LAST_RESULTS
    if "nc" not in _CACHE:
        _CACHE["nc"] = _build_module()
    nc = _CACHE["nc"]

    in_maps = _prep_inputs(x, condition, w1, b1, w2, b2,
                           kernels_weights, kernels_bias)

    res = run_bass_kernel_spmd(nc, in_maps, core_ids=list(range(NCORES)))
    LAST_RESULTS = res

    out = np.concatenate([res.results[c]["y"] for c in range(NCORES)], axis=1)
    return np.ascontiguousarray(out, dtype=np.float32)


if __name__ == "__main__":
    rng = np.random.default_rng(0)
    ins = {
        "x": rng.standard_normal((B, IN), dtype=np.float32),
        "condition": rng.standard_normal((1, IN), dtype=np.float32),
        "w1": rng.standard_normal((IN, H), dtype=np.float32) * 0.02,
        "b1": np.zeros(H, np.float32),
        "w2": rng.standard_normal((H, K), dtype=np.float32) * 0.02,
        "b2": np.zeros(K, np.float32),
        "kernels_weights": rng.standard_normal((K, OUT, IN),
                                               dtype=np.float32) * 0.01,
        "kernels_bias": np.zeros((K, OUT), np.float32),
    }
    y = kernel(**ins)
    print("out", y.shape, y.dtype, float(np.abs(y).mean()))


# revision 6
# speedup vs baseline: 1.1444x; 1.1444x over previous
"""Trainium2 Bass kernel for DynamicCondLinear (MoE-routing style).

Math: condition batch is 1, so the softmax routing weights (K=8) are shared by
all 32 samples; out = sum_k a_k * (x @ W_k^T) + sum_k a_k * b_k with
a = softmax(relu(cond @ w1 + b1) @ w2 + b2).

Sharding: tensor-parallel over OUT channels (2048 / 8 cores = 256 per core).
Each core streams its 8 MiB fp16 weight shard from HBM once; that stream is
the roofline (~20 us at the 412 GB/s measured 8KiB-packet rate).

v2 schedule (trace-driven redesign of the 43.3 us baseline):
 - per-k PSUM groups: slab k's matmuls accumulate raw x @ W_k^T into a
   dedicated (B, OC) PSUM slice, so the main contraction starts as soon as
   x + slab 0 land (~4 us) and never waits on the alpha MLP. The e-weighted
   combine runs as one fused DVE op per k (scalar_tensor_tensor:
   acc' = psum_k * e_k + acc), staggered with slab completions; only the
   k=7 op is on the tail.
 - single bulk HWDGE queue: concurrent queues measurably interfere (~337
   GB/s combined vs ~410 alone), so all streaming rides the sync ring with
   >=2 KiB descriptors; the scalar ring only carries one output half at the
   end. Last slab lands in 4 column quarters to keep the PE trailing the
   final bytes by <0.5 us.
 - w1 ships as fp8e4m3 (1 MiB instead of 2 MiB bf16): measured end-to-end
   rel err 6.9e-3 vs the 2e-2 gate. The PE upconverts fp8 rhs against the
   fp16 condition stationary.
 - softmax normalization is folded into the routing weights (e_k / sum(e))
   before the combine, so the output needs no final rescale pass.
 - dependency-free warm-up matmuls hold the PE HAM clock up through the DMA
   prefix; a tail batch keeps the sequencer hot into the NEFF epilogue's
   distributed semaphore clear (which otherwise runs at the LOW p-state).

Host-side prep is layout-only (transpose/reshape/cast for DMA-friendly
tiling); all math happens on-device.
"""

import os
import sys

import numpy as np

if "/opt/trn_rl_repo" not in sys.path:
    sys.path.insert(0, "/opt/trn_rl_repo")

import concourse.bacc as bacc
import concourse.mybir as mybir
import concourse.tile as tile
from concourse.bass_utils import run_bass_kernel_spmd

B, IN, OUT, K, H = 32, 2048, 2048, 8, 512
NCORES = 8
OC = OUT // NCORES  # 256 out channels per core
JT = IN // 128      # 16 contraction tiles
HT = H // 128       # 4 hidden tiles

F32 = mybir.dt.float32
FP16 = mybir.dt.float16
FP8 = mybir.dt.float8e4

CXW = JT + JT * B + HT * K   # fp16 pack: ct | xt | w2t  (560 cols)
XOFF = JT
WOFF = JT + JT * B

_CACHE = {}
LAST_RESULTS = None  # test.py reads this for profiling info


def _build_module():
    nc = bacc.Bacc("TRN2", target_bir_lowering=False, debug=False,
                   num_devices=NCORES)

    wt_d = nc.dram_tensor("wt", (K, 128, JT * OC), FP16, kind="ExternalInput")
    cxw_d = nc.dram_tensor("cxw", (128, CXW), FP16, kind="ExternalInput")
    w18_d = nc.dram_tensor("w18", (128, JT * H), FP8, kind="ExternalInput")
    b1r_d = nc.dram_tensor("b1r", (1, H), FP16, kind="ExternalInput")
    b2r_d = nc.dram_tensor("b2r", (1, K), FP16, kind="ExternalInput")
    kb_d = nc.dram_tensor("kb", (K, OC), FP16, kind="ExternalInput")
    y_d = nc.dram_tensor("y", (B, OC), F32, kind="ExternalOutput")
    # warmup sink: consumed so bacc's DCE keeps the PE warm-up matmuls
    ysink_d = nc.dram_tensor("ysink", (1, 1), F32, kind="ExternalOutput")

    n_warm1 = int(os.environ.get("KERNEL_WARMUP1", "12"))
    n_tailwarm = int(os.environ.get("KERNEL_TAILWARM", "28"))

    with tile.TileContext(nc) as tc:
        with (
            tc.tile_pool(name="cpool", bufs=1) as cpool,
            tc.tile_pool(name="wpool", bufs=1) as wpool,
            tc.tile_pool(name="ppool", bufs=1, space="PSUM") as ppool,
        ):
            # --- DMA: ONE bulk queue (sync). A measured lesson: running
            # both HWDGE queues concurrently drops combined throughput to
            # ~337 GB/s vs ~410 single-queue (per-packet round-robin
            # interference), so everything big goes on sync, ordered by
            # need: x/condition pack first (stationaries), then slabs,
            # with the alpha-MLP weights mid-stream (the per-k PSUM design
            # only needs `e` before the final combine). Slab 7 lands in 4
            # column quarters for a tight tail. ---
            slabs = [wpool.tile((128, JT * OC), FP16, tag="wt_slab", bufs=K,
                                name=f"wt_slab{k}")
                     for k in range(K)]
            cxw_sb = cpool.tile((128, CXW), FP16)
            w18_sb = cpool.tile((128, JT * H), FP8)
            kb_sb = cpool.tile((K, OC), FP16)
            b1r_sb = cpool.tile((1, H), FP16)
            b2r_sb = cpool.tile((1, K), FP16)

            nc.sync.dma_start(cxw_sb[:], cxw_d.ap())
            for k in (0, 1, 2):
                nc.sync.dma_start(slabs[k][:], wt_d.ap()[k])
            WC = JT * H // 4  # 2048 cols per w1 chunk
            for c in range(4):
                nc.sync.dma_start(w18_sb[:, c * WC:(c + 1) * WC],
                                  w18_d.ap()[:, c * WC:(c + 1) * WC])
            nc.sync.dma_start(kb_sb[:], kb_d.ap())
            nc.sync.dma_start(b1r_sb[:], b1r_d.ap())
            nc.sync.dma_start(b2r_sb[:], b2r_d.ap())
            for k in (3, 4, 5, 6):
                nc.sync.dma_start(slabs[k][:], wt_d.ap()[k])
            QC = JT * OC // 4  # 1024 cols per quarter
            for q in range(4):
                nc.sync.dma_start(slabs[7][:, q * QC:(q + 1) * QC],
                                  wt_d.ap()[7][:, q * QC:(q + 1) * QC])

            # --- constants ---
            one1h = cpool.tile((1, 1), FP16)
            nc.gpsimd.memset(one1h[:], 1.0)
            ones_b = cpool.tile((1, B), FP16)
            nc.gpsimd.memset(ones_b[:], 1.0)

            # --- PE warm-up: dependency-free matmuls ramp the HAM clock ---
            dum_a = cpool.tile((128, B), FP16)
            nc.gpsimd.memset(dum_a[:], 0.0)
            dum_b = cpool.tile((128, OC), FP16)
            nc.gpsimd.memset(dum_b[:], 0.0)
            dum_psum = ppool.tile((B, OC), F32)
            dum_sink = cpool.tile((1, 1), F32)

            def warmup(n):
                for _ in range(n):
                    nc.tensor.matmul(dum_psum[:], dum_a[:], dum_b[:],
                                     start=True, stop=True)

            warmup(n_warm1)

            # --- main contraction: one PSUM slice per k, raw x @ W_k^T.
            # PE program order interleaves slab groups with the MLP so
            # whichever data lands first keeps the PE busy. ---
            mpsum = ppool.tile((B, K * OC), F32)

            def slab_group(k):
                for j in range(JT):
                    nc.tensor.matmul(
                        mpsum[:, k * OC:(k + 1) * OC],
                        cxw_sb[:, XOFF + j * B:XOFF + (j + 1) * B],
                        slabs[k][:, j * OC:(j + 1) * OC],
                        start=(j == 0), stop=(j == JT - 1),
                    )

            slab_group(0)

            # --- alpha MLP: h = relu(cond @ w1 + b1), fp8 weights ---
            psum_h = ppool.tile((1, H), F32, tag="pA")
            for t in range(JT):
                nc.tensor.matmul(
                    psum_h[:],
                    cxw_sb[:, t:t + 1],                  # ct column t
                    w18_sb[:, t * H:(t + 1) * H],        # w1 fp8 slab t
                    start=(t == 0), stop=False,
                )
            nc.tensor.matmul(psum_h[:], one1h[:], b1r_sb[:],
                             start=False, stop=True)
            h_sb = cpool.tile((1, H), FP16)
            nc.scalar.activation(h_sb[:], psum_h[:],
                                 mybir.ActivationFunctionType.Relu)

            slab_group(1)

            # transpose h (1,512) -> hT (128,4) via tiny matmuls vs ones
            psum_ht = ppool.tile((128, HT), F32, tag="pB")
            for q in range(HT):
                nc.tensor.matmul(
                    psum_ht[:, q:q + 1],
                    h_sb[:, q * 128:(q + 1) * 128],
                    one1h[:],
                    start=True, stop=True,
                )
            ht_sb = cpool.tile((128, HT), FP16)
            nc.vector.tensor_copy(ht_sb[:], psum_ht[:])

            # scores row (1, 8) = sum_q hT[:,q].T @ w2t[:,q,:] + b2
            psum_s = ppool.tile((1, K), F32, tag="pC")
            for q in range(HT):
                nc.tensor.matmul(
                    psum_s[:],
                    ht_sb[:, q:q + 1],
                    cxw_sb[:, WOFF + q * K:WOFF + (q + 1) * K],
                    start=(q == 0), stop=False,
                )
            nc.tensor.matmul(psum_s[:], one1h[:], b2r_sb[:],
                             start=False, stop=True)

            slab_group(2)

            # softmax, normalized up front: en = exp(s) / sum(exp(s));
            # no max-subtraction (scores are O(1) for this model family)
            e_sb = cpool.tile((1, K), F32)
            nc.scalar.activation(e_sb[:], psum_s[:],
                                 mybir.ActivationFunctionType.Exp)
            esum = cpool.tile((1, 1), F32)
            nc.vector.reduce_sum(esum[:], e_sb[:], axis=mybir.AxisListType.X)
            rinv = cpool.tile((1, 1), F32)
            nc.vector.reciprocal(rinv[:], esum[:])
            en16 = cpool.tile((1, K), FP16)
            nc.vector.tensor_scalar_mul(en16[:], e_sb[:], rinv[:])

            # broadcast en to all B partitions (for the combine scalars)
            psum_e = ppool.tile((B, K), F32, tag="pB")
            nc.tensor.matmul(psum_e[:], ones_b[:], en16[:],
                             start=True, stop=True)
            e32_sb = cpool.tile((B, K), F32)
            nc.vector.tensor_copy(e32_sb[:], psum_e[:])

            # en column (8,1) -> e-weighted bias row (1, OC) -> broadcast
            # to (B, OC): the combine chain's initial accumulator
            psum_ac = ppool.tile((K, 1), F32, tag="pA")
            nc.tensor.matmul(psum_ac[:], en16[:], one1h[:],
                             start=True, stop=True)
            e_c = cpool.tile((K, 1), FP16)
            nc.vector.tensor_copy(e_c[:], psum_ac[:])
            psum_bb = ppool.tile((1, OC), F32, tag="pC")
            nc.tensor.matmul(psum_bb[:], e_c[:], kb_sb[:],
                             start=True, stop=True)
            aggb_sb = cpool.tile((1, OC), FP16)
            nc.vector.tensor_copy(aggb_sb[:], psum_bb[:])
            psum_cb = ppool.tile((B, OC), F32, tag="pA")
            nc.tensor.matmul(psum_cb[:], ones_b[:], aggb_sb[:],
                             start=True, stop=True)

            for k in range(3, K):
                slab_group(k)

            # --- tail warm: keep the PE sequencer at speed through the
            # NEFF epilogue's distributed semaphore clear ---
            warmup(n_tailwarm)
            nc.vector.tensor_copy(dum_sink[:], dum_psum[0:1, 0:1])
            nc.scalar.dma_start(ysink_d.ap(), dum_sink[:])

            # --- combine: acc' = psum_k * en_k + acc, one fused DVE op
            # per k; k<7 complete during the stream, only k=7 is tail.
            # The last combine splits into OC halves so each output half
            # DMAs (on its own queue) as soon as its half is done. ---
            acc_a = cpool.tile((B, OC), F32)
            acc_b = cpool.tile((B, OC), F32)
            y_sb = cpool.tile((B, OC), F32)
            nc.vector.tensor_copy(acc_a[:], psum_cb[:])
            cur, nxt = acc_a, acc_b
            for k in range(K - 1):
                nc.vector.scalar_tensor_tensor(
                    nxt[:],
                    mpsum[:, k * OC:(k + 1) * OC],
                    e32_sb[:, k:k + 1],
                    cur[:],
                    op0=mybir.AluOpType.mult,
                    op1=mybir.AluOpType.add,
                )
                cur, nxt = nxt, cur
            HOC = OC // 2
            for h, eng in ((0, nc.scalar), (1, nc.sync)):
                sl = slice(h * HOC, (h + 1) * HOC)
                nc.vector.scalar_tensor_tensor(
                    y_sb[:, sl],
                    mpsum[:, (K - 1) * OC + h * HOC:(K - 1) * OC + (h + 1) * HOC],
                    e32_sb[:, K - 1:K],
                    cur[:, sl],
                    op0=mybir.AluOpType.mult,
                    op1=mybir.AluOpType.add,
                )
                eng.dma_start(y_d.ap()[:, sl], y_sb[:, sl])

    nc.compile()
    return nc


def _prep_inputs(x, condition, w1, b1, w2, b2, kernels_weights, kernels_bias):
    """Layout-only host prep: slice per-core shards and retile for DMA."""
    import ml_dtypes
    f = np.float32
    f16 = np.float16
    f8 = ml_dtypes.float8_e4m3
    x = np.asarray(x, f)
    condition = np.asarray(condition, f)
    w1 = np.asarray(w1, f)
    b1 = np.asarray(b1, f)
    w2 = np.asarray(w2, f)
    b2 = np.asarray(b2, f)
    kernels_weights = np.asarray(kernels_weights, f)
    kernels_bias = np.asarray(kernels_bias, f)

    # xT tiled: xt[p, j*B + b] = x[b, j*128 + p]
    xt = np.ascontiguousarray(
        x.T.reshape(JT, 128, B).transpose(1, 0, 2)).reshape(128, JT * B)
    # w2 tiled as rhs: w2t[p, q*K + k] = w2[q*128 + p, k]
    w2t = np.ascontiguousarray(
        w2.reshape(HT, 128, K).transpose(1, 0, 2)).reshape(128, HT * K)
    ct = np.ascontiguousarray(condition.reshape(JT, 128).T)  # (128, JT)
    cxw = np.concatenate([ct, xt, w2t], axis=1).astype(f16)
    cxw = np.ascontiguousarray(cxw)

    # w1 tiled fp8: w18[p, t*H + h] = w1[t*128 + p, h]
    w18 = np.ascontiguousarray(
        w1.reshape(JT, 128, H).transpose(1, 0, 2)).reshape(128, JT * H)
    w18 = w18.astype(f8)

    b1r = np.ascontiguousarray(b1.reshape(1, H)).astype(f16)
    b2r = np.ascontiguousarray(b2.reshape(1, K)).astype(f16)

    in_maps = []
    for c in range(NCORES):
        osl = slice(c * OC, (c + 1) * OC)
        # W shard [k, o, i] -> tiles [k, p, j, o] with i = j*128 + p
        wt = np.ascontiguousarray(
            kernels_weights[:, osl, :].reshape(K, OC, JT, 128)
            .transpose(0, 3, 2, 1)).reshape(K, 128, JT * OC).astype(f16)
        kb = np.ascontiguousarray(kernels_bias[:, osl]).astype(f16)
        in_maps.append({
            "wt": wt, "cxw": cxw, "w18": w18,
            "b1r": b1r, "b2r": b2r, "kb": kb,
        })
    return in_maps


def kernel(x, condition, w1, b1, w2, b2, kernels_weights, kernels_bias):
    global LAST_RESULTS
    if "nc" not in _CACHE:
        _CACHE["nc"] = _build_module()
    nc = _CACHE["nc"]

    in_maps = _prep_inputs(x, condition, w1, b1, w2, b2,
                           kernels_weights, kernels_bias)

    res = run_bass_kernel_spmd(nc, in_maps, core_ids=list(range(NCORES)))
    LAST_RESULTS = res

    out = np.concatenate([res.results[c]["y"] for c in range(NCORES)], axis=1)
    return np.ascontiguousarray(out, dtype=np.float32)


if __name__ == "__main__":
    rng = np.random.default_rng(0)
    ins = {
        "x": rng.standard_normal((B, IN), dtype=np.float32),
        "condition": rng.standard_normal((1, IN), dtype=np.float32),
        "w1": rng.standard_normal((IN, H), dtype=np.float32) * 0.02,
        "b1": np.zeros(H, np.float32),
        "w2": rng.standard_normal((H, K), dtype=np.float32) * 0.02,
        "b2": np.zeros(K, np.float32),
        "kernels_weights": rng.standard_normal((K, OUT, IN),
                                               dtype=np.float32) * 0.01,
        "kernels_bias": np.zeros((K, OUT), np.float32),
    }
    y = kernel(**ins)
    print("out", y.shape, y.dtype, float(np.abs(y).mean()))


# revision 11
# speedup vs baseline: 1.3778x; 1.2039x over previous
"""Trainium2 Bass kernel for DynamicCondLinear (MoE-routing style).

Math: condition batch is 1, so the softmax routing weights (K=8) are shared by
all 32 samples; out = sum_k a_k * (x @ W_k^T) + sum_k a_k * b_k with
a = softmax(relu(cond @ w1 + b1) @ w2 + b2).

Sharding: tensor-parallel over OUT channels (2048 / 8 cores = 256 per core).
Each core streams its 8 MiB fp16 weight shard from HBM once; that stream is
the roofline (~20 us at the 412 GB/s measured 8KiB-packet rate).

v2 schedule (trace-driven redesign of the 43.3 us baseline):
 - per-k PSUM groups: slab k's matmuls accumulate raw x @ W_k^T into a
   dedicated (B, OC) PSUM slice, so the main contraction starts as soon as
   x + slab 0 land (~4 us) and never waits on the alpha MLP. The e-weighted
   combine runs as one fused DVE op per k (scalar_tensor_tensor:
   acc' = psum_k * e_k + acc), staggered with slab completions; only the
   k=7 op is on the tail.
 - single bulk HWDGE queue: concurrent queues measurably interfere (~337
   GB/s combined vs ~410 alone), so all streaming rides the sync ring with
   >=2 KiB descriptors; the scalar ring only carries one output half at the
   end. Last slab lands in 4 column quarters to keep the PE trailing the
   final bytes by <0.5 us.
 - w1 ships as fp8e4m3 (1 MiB instead of 2 MiB bf16): measured end-to-end
   rel err 6.9e-3 vs the 2e-2 gate. The PE upconverts fp8 rhs against the
   fp16 condition stationary.
 - softmax normalization is folded into the routing weights (e_k / sum(e))
   before the combine, so the output needs no final rescale pass.
 - dependency-free warm-up matmuls hold the PE HAM clock up through the DMA
   prefix; a tail batch keeps the sequencer hot into the NEFF epilogue's
   distributed semaphore clear (which otherwise runs at the LOW p-state).

Host-side prep is layout-only (transpose/reshape/cast for DMA-friendly
tiling); all math happens on-device.
"""

import os
import sys

import numpy as np

if "/opt/trn_rl_repo" not in sys.path:
    sys.path.insert(0, "/opt/trn_rl_repo")

import concourse.bacc as bacc
import concourse.mybir as mybir
import concourse.tile as tile
from concourse.bass_utils import run_bass_kernel_spmd

B, IN, OUT, K, H = 32, 2048, 2048, 8, 512
NCORES = 8
OC = OUT // NCORES  # 256 out channels per core
JT = IN // 128      # 16 contraction tiles
HT = H // 128       # 4 hidden tiles

F32 = mybir.dt.float32
FP16 = mybir.dt.float16
FP8 = mybir.dt.float8e4

CXW = JT + JT * B + HT * K   # fp16 pack: ct | xt | w2t  (560 cols)
XOFF = JT
WOFF = JT + JT * B

_CACHE = {}
LAST_RESULTS = None  # test.py reads this for profiling info


def _build_module():
    nc = bacc.Bacc("TRN2", target_bir_lowering=False, debug=False,
                   num_devices=NCORES)

    wt_d = nc.dram_tensor("wt", (K, 128, JT * OC), FP16, kind="ExternalInput")
    cxw_d = nc.dram_tensor("cxw", (128, CXW), FP16, kind="ExternalInput")
    w18_d = nc.dram_tensor("w18", (128, JT * H), FP8, kind="ExternalInput")
    b1r_d = nc.dram_tensor("b1r", (1, H), FP16, kind="ExternalInput")
    b2r_d = nc.dram_tensor("b2r", (1, K), FP16, kind="ExternalInput")
    kb_d = nc.dram_tensor("kb", (K, OC), FP16, kind="ExternalInput")
    y_d = nc.dram_tensor("y", (B, OC), F32, kind="ExternalOutput")
    # warmup sink: consumed so bacc's DCE keeps the PE warm-up matmuls
    ysink_d = nc.dram_tensor("ysink", (1, 1), F32, kind="ExternalOutput")

    n_warm1 = int(os.environ.get("KERNEL_WARMUP1", "12"))
    n_tailwarm = int(os.environ.get("KERNEL_TAILWARM", "28"))

    with tile.TileContext(nc) as tc:
        with (
            tc.tile_pool(name="cpool", bufs=1) as cpool,
            tc.tile_pool(name="wpool", bufs=1) as wpool,
            tc.tile_pool(name="ppool", bufs=1, space="PSUM") as ppool,
        ):
            # --- DMA: ONE bulk queue (sync). A measured lesson: running
            # both HWDGE queues concurrently drops combined throughput to
            # ~337 GB/s vs ~410 single-queue (per-packet round-robin
            # interference), so everything big goes on sync, ordered by
            # need: x/condition pack first (stationaries), then slabs,
            # with the alpha-MLP weights mid-stream (the per-k PSUM design
            # only needs `e` before the final combine). Slab 7 lands in 4
            # column quarters for a tight tail. ---
            slabs = [wpool.tile((128, JT * OC), FP16, tag="wt_slab", bufs=K,
                                name=f"wt_slab{k}")
                     for k in range(K)]
            cxw_sb = cpool.tile((128, CXW), FP16)
            w18_sb = cpool.tile((128, JT * H), FP8)
            kb_sb = cpool.tile((K, OC), FP16)
            b1r_sb = cpool.tile((1, H), FP16)
            b2r_sb = cpool.tile((1, K), FP16)

            # w1-fp8 chunks ride between slabs so the queue always has >=2
            # bulk transfers pending (a cluster of small DMAs serializes on
            # the framework's DMA-semaphore-reuse waits and drains the
            # stream). The 3 tiny bias loads go on the otherwise-idle
            # scalar ring for the same reason.
            WC = JT * H // 4  # 2048 cols per w1 chunk
            nc.sync.dma_start(cxw_sb[:], cxw_d.ap())
            nc.sync.dma_start(slabs[0][:], wt_d.ap()[0])
            for c in range(4):
                nc.sync.dma_start(w18_sb[:, c * WC:(c + 1) * WC],
                                  w18_d.ap()[:, c * WC:(c + 1) * WC])
                nc.sync.dma_start(slabs[c + 1][:], wt_d.ap()[c + 1])
            for k in (5, 6):
                nc.sync.dma_start(slabs[k][:], wt_d.ap()[k])
            QC = JT * OC // 4  # 1024 cols per quarter
            for q in range(4):
                nc.sync.dma_start(slabs[7][:, q * QC:(q + 1) * QC],
                                  wt_d.ap()[7][:, q * QC:(q + 1) * QC])
            nc.scalar.dma_start(kb_sb[:], kb_d.ap())
            nc.scalar.dma_start(b1r_sb[:], b1r_d.ap())
            nc.scalar.dma_start(b2r_sb[:], b2r_d.ap())

            # --- constants ---
            one1h = cpool.tile((1, 1), FP16)
            nc.gpsimd.memset(one1h[:], 1.0)
            ones_b = cpool.tile((1, B), FP16)
            nc.gpsimd.memset(ones_b[:], 1.0)

            # --- PE warm-up: dependency-free matmuls ramp the HAM clock ---
            dum_a = cpool.tile((128, B), FP16)
            nc.gpsimd.memset(dum_a[:], 0.0)
            dum_b = cpool.tile((128, OC), FP16)
            nc.gpsimd.memset(dum_b[:], 0.0)
            dum_psum = ppool.tile((B, OC), F32)
            dum_sink = cpool.tile((1, 1), F32)

            def warmup(n):
                for _ in range(n):
                    nc.tensor.matmul(dum_psum[:], dum_a[:], dum_b[:],
                                     start=True, stop=True)

            warmup(n_warm1)

            # --- main contraction: raw x @ W_k^T per k. PSUM deps are
            # tile-granular and banks are 2 KB, so k=0..5 pair up into
            # three (B, 2*OC) tiles (their combines are e-gated anyway)
            # while k=6,7 get their own tiles to keep the tail short. PE
            # program order interleaves slab groups with the MLP so
            # whichever data lands first keeps the PE busy. ---
            mp01 = ppool.tile((B, 2 * OC), F32)
            mp23 = ppool.tile((B, 2 * OC), F32)
            mp45 = ppool.tile((B, 2 * OC), F32)
            mp6 = ppool.tile((B, OC), F32)
            mp7 = ppool.tile((B, OC), F32)

            def mp_ap(k):
                if k >= 6:
                    return (mp6 if k == 6 else mp7)[:]
                return (mp01, mp23, mp45)[k // 2][:, (k % 2) * OC:
                                                  (k % 2 + 1) * OC]

            def slab_group(k):
                for j in range(JT):
                    nc.tensor.matmul(
                        mp_ap(k),
                        cxw_sb[:, XOFF + j * B:XOFF + (j + 1) * B],
                        slabs[k][:, j * OC:(j + 1) * OC],
                        start=(j == 0), stop=(j == JT - 1),
                    )

            slab_group(0)

            # --- alpha MLP: h = relu(cond @ w1 + b1), fp8 weights ---
            psum_h = ppool.tile((1, H), F32, tag="mlp")
            for t in range(JT):
                nc.tensor.matmul(
                    psum_h[:],
                    cxw_sb[:, t:t + 1],                  # ct column t
                    w18_sb[:, t * H:(t + 1) * H],        # w1 fp8 slab t
                    start=(t == 0), stop=False,
                )
            nc.tensor.matmul(psum_h[:], one1h[:], b1r_sb[:],
                             start=False, stop=True)
            h_sb = cpool.tile((1, H), FP16)
            nc.scalar.activation(h_sb[:], psum_h[:],
                                 mybir.ActivationFunctionType.Relu)

            slab_group(1)

            # transpose h (1,512) -> hT (128,4) via tiny matmuls vs ones
            psum_ht = ppool.tile((128, HT), F32, tag="mlp")
            for q in range(HT):
                nc.tensor.matmul(
                    psum_ht[:, q:q + 1],
                    h_sb[:, q * 128:(q + 1) * 128],
                    one1h[:],
                    start=True, stop=True,
                )
            ht_sb = cpool.tile((128, HT), FP16)
            nc.vector.tensor_copy(ht_sb[:], psum_ht[:])

            # scores row (1, 8) = sum_q hT[:,q].T @ w2t[:,q,:] + b2
            psum_s = ppool.tile((1, K), F32, tag="mlp")
            for q in range(HT):
                nc.tensor.matmul(
                    psum_s[:],
                    ht_sb[:, q:q + 1],
                    cxw_sb[:, WOFF + q * K:WOFF + (q + 1) * K],
                    start=(q == 0), stop=False,
                )
            nc.tensor.matmul(psum_s[:], one1h[:], b2r_sb[:],
                             start=False, stop=True)

            slab_group(2)

            # softmax, normalized up front: en = exp(s) / sum(exp(s));
            # no max-subtraction (scores are O(1) for this model family)
            e_sb = cpool.tile((1, K), F32)
            nc.scalar.activation(e_sb[:], psum_s[:],
                                 mybir.ActivationFunctionType.Exp)
            esum = cpool.tile((1, 1), F32)
            nc.vector.reduce_sum(esum[:], e_sb[:], axis=mybir.AxisListType.X)
            rinv = cpool.tile((1, 1), F32)
            nc.vector.reciprocal(rinv[:], esum[:])
            en16 = cpool.tile((1, K), FP16)
            nc.vector.tensor_scalar_mul(en16[:], e_sb[:], rinv[:])

            # broadcast en to all B partitions (for the combine scalars)
            psum_e = ppool.tile((B, K), F32, tag="mlp")
            nc.tensor.matmul(psum_e[:], ones_b[:], en16[:],
                             start=True, stop=True)
            e32_sb = cpool.tile((B, K), F32)
            nc.vector.tensor_copy(e32_sb[:], psum_e[:])

            # en column (8,1) -> e-weighted bias row (1, OC) -> broadcast
            # to (B, OC): the combine chain's initial accumulator
            psum_ac = ppool.tile((K, 1), F32, tag="mlp")
            nc.tensor.matmul(psum_ac[:], en16[:], one1h[:],
                             start=True, stop=True)
            e_c = cpool.tile((K, 1), FP16)
            nc.vector.tensor_copy(e_c[:], psum_ac[:])
            psum_bb = ppool.tile((1, OC), F32, tag="mlp")
            nc.tensor.matmul(psum_bb[:], e_c[:], kb_sb[:],
                             start=True, stop=True)
            aggb_sb = cpool.tile((1, OC), FP16)
            nc.vector.tensor_copy(aggb_sb[:], psum_bb[:])
            psum_cb = ppool.tile((B, OC), F32, tag="mlp")
            nc.tensor.matmul(psum_cb[:], ones_b[:], aggb_sb[:],
                             start=True, stop=True)

            for k in range(3, K):
                slab_group(k)

            # --- tail warm: keep the PE sequencer at speed through the
            # NEFF epilogue's distributed semaphore clear ---
            warmup(n_tailwarm)
            nc.vector.tensor_copy(dum_sink[:], dum_psum[0:1, 0:1])
            nc.scalar.dma_start(ysink_d.ap(), dum_sink[:])

            # --- combine: acc' = psum_k * en_k + acc, one fused DVE op
            # per k; k<7 complete during the stream, only k=7 is tail.
            # The last combine splits into OC halves so each output half
            # DMAs (on its own queue) as soon as its half is done. ---
            acc_a = cpool.tile((B, OC), F32)
            acc_b = cpool.tile((B, OC), F32)
            y_sb = cpool.tile((B, OC), F32)
            nc.vector.tensor_copy(acc_a[:], psum_cb[:])
            cur, nxt = acc_a, acc_b
            for k in range(K - 1):
                nc.vector.scalar_tensor_tensor(
                    nxt[:],
                    mp_ap(k),
                    e32_sb[:, k:k + 1],
                    cur[:],
                    op0=mybir.AluOpType.mult,
                    op1=mybir.AluOpType.add,
                )
                cur, nxt = nxt, cur
            HOC = OC // 2
            for h, eng in ((0, nc.scalar), (1, nc.sync)):
                sl = slice(h * HOC, (h + 1) * HOC)
                nc.vector.scalar_tensor_tensor(
                    y_sb[:, sl],
                    mp7[:, sl],
                    e32_sb[:, K - 1:K],
                    cur[:, sl],
                    op0=mybir.AluOpType.mult,
                    op1=mybir.AluOpType.add,
                )
                eng.dma_start(y_d.ap()[:, sl], y_sb[:, sl])

    nc.compile()
    return nc


def _prep_inputs(x, condition, w1, b1, w2, b2, kernels_weights, kernels_bias):
    """Layout-only host prep: slice per-core shards and retile for DMA."""
    import ml_dtypes
    f = np.float32
    f16 = np.float16
    f8 = ml_dtypes.float8_e4m3
    x = np.asarray(x, f)
    condition = np.asarray(condition, f)
    w1 = np.asarray(w1, f)
    b1 = np.asarray(b1, f)
    w2 = np.asarray(w2, f)
    b2 = np.asarray(b2, f)
    kernels_weights = np.asarray(kernels_weights, f)
    kernels_bias = np.asarray(kernels_bias, f)

    # xT tiled: xt[p, j*B + b] = x[b, j*128 + p]
    xt = np.ascontiguousarray(
        x.T.reshape(JT, 128, B).transpose(1, 0, 2)).reshape(128, JT * B)
    # w2 tiled as rhs: w2t[p, q*K + k] = w2[q*128 + p, k]
    w2t = np.ascontiguousarray(
        w2.reshape(HT, 128, K).transpose(1, 0, 2)).reshape(128, HT * K)
    ct = np.ascontiguousarray(condition.reshape(JT, 128).T)  # (128, JT)
    cxw = np.concatenate([ct, xt, w2t], axis=1).astype(f16)
    cxw = np.ascontiguousarray(cxw)

    # w1 tiled fp8: w18[p, t*H + h] = w1[t*128 + p, h]
    w18 = np.ascontiguousarray(
        w1.reshape(JT, 128, H).transpose(1, 0, 2)).reshape(128, JT * H)
    w18 = w18.astype(f8)

    b1r = np.ascontiguousarray(b1.reshape(1, H)).astype(f16)
    b2r = np.ascontiguousarray(b2.reshape(1, K)).astype(f16)

    in_maps = []
    for c in range(NCORES):
        osl = slice(c * OC, (c + 1) * OC)
        # W shard [k, o, i] -> tiles [k, p, j, o] with i = j*128 + p
        wt = np.ascontiguousarray(
            kernels_weights[:, osl, :].reshape(K, OC, JT, 128)
            .transpose(0, 3, 2, 1)).reshape(K, 128, JT * OC).astype(f16)
        kb = np.ascontiguousarray(kernels_bias[:, osl]).astype(f16)
        in_maps.append({
            "wt": wt, "cxw": cxw, "w18": w18,
            "b1r": b1r, "b2r": b2r, "kb": kb,
        })
    return in_maps


def kernel(x, condition, w1, b1, w2, b2, kernels_weights, kernels_bias):
    global LAST_RESULTS
    if "nc" not in _CACHE:
        _CACHE["nc"] = _build_module()
    nc = _CACHE["nc"]

    in_maps = _prep_inputs(x, condition, w1, b1, w2, b2,
                           kernels_weights, kernels_bias)

    res = run_bass_kernel_spmd(nc, in_maps, core_ids=list(range(NCORES)))
    LAST_RESULTS = res

    out = np.concatenate([res.results[c]["y"] for c in range(NCORES)], axis=1)
    return np.ascontiguousarray(out, dtype=np.float32)


if __name__ == "__main__":
    rng = np.random.default_rng(0)
    ins = {
        "x": rng.standard_normal((B, IN), dtype=np.float32),
        "condition": rng.standard_normal((1, IN), dtype=np.float32),
        "w1": rng.standard_normal((IN, H), dtype=np.float32) * 0.02,
        "b1": np.zeros(H, np.float32),
        "w2": rng.standard_normal((H, K), dtype=np.float32) * 0.02,
        "b2": np.zeros(K, np.float32),
        "kernels_weights": rng.standard_normal((K, OUT, IN),
                                               dtype=np.float32) * 0.01,
        "kernels_bias": np.zeros((K, OUT), np.float32),
    }
    y = kernel(**ins)
    print("out", y.shape, y.dtype, float(np.abs(y).mean()))


# revision 13
# speedup vs baseline: 1.4184x; 1.0295x over previous
"""Trainium2 Bass kernel for DynamicCondLinear (MoE-routing style).

Math: condition batch is 1, so the softmax routing weights (K=8) are shared by
all 32 samples; out = sum_k a_k * (x @ W_k^T) + sum_k a_k * b_k with
a = softmax(relu(cond @ w1 + b1) @ w2 + b2).

Sharding: tensor-parallel over OUT channels (2048 / 8 cores = 256 per core).
Each core streams its 8 MiB fp16 weight shard from HBM once; that stream is
the roofline (~20 us at the 412 GB/s measured 8KiB-packet rate).

v2 schedule (trace-driven redesign of the 43.3 us baseline):
 - per-k PSUM groups: slab k's matmuls accumulate raw x @ W_k^T into a
   dedicated (B, OC) PSUM slice, so the main contraction starts as soon as
   x + slab 0 land (~4 us) and never waits on the alpha MLP. The e-weighted
   combine runs as one fused DVE op per k (scalar_tensor_tensor:
   acc' = psum_k * e_k + acc), staggered with slab completions; only the
   k=7 op is on the tail.
 - single bulk HWDGE queue: concurrent queues measurably interfere (~337
   GB/s combined vs ~410 alone), so all streaming rides the sync ring with
   >=2 KiB descriptors; the scalar ring only carries one output half at the
   end. Last slab lands in 4 column quarters to keep the PE trailing the
   final bytes by <0.5 us.
 - w1 ships as fp8e4m3 (1 MiB instead of 2 MiB bf16): measured end-to-end
   rel err 6.9e-3 vs the 2e-2 gate. The PE upconverts fp8 rhs against the
   fp16 condition stationary.
 - softmax normalization is folded into the routing weights (e_k / sum(e))
   before the combine, so the output needs no final rescale pass.
 - dependency-free warm-up matmuls hold the PE HAM clock up through the DMA
   prefix; a tail batch keeps the sequencer hot into the NEFF epilogue's
   distributed semaphore clear (which otherwise runs at the LOW p-state).

Host-side prep is layout-only (transpose/reshape/cast for DMA-friendly
tiling); all math happens on-device.
"""

import os
import sys

import numpy as np

if "/opt/trn_rl_repo" not in sys.path:
    sys.path.insert(0, "/opt/trn_rl_repo")

import concourse.bacc as bacc
import concourse.mybir as mybir
import concourse.tile as tile
from concourse.bass_utils import run_bass_kernel_spmd

B, IN, OUT, K, H = 32, 2048, 2048, 8, 512
NCORES = 8
OC = OUT // NCORES  # 256 out channels per core
JT = IN // 128      # 16 contraction tiles
HT = H // 128       # 4 hidden tiles

F32 = mybir.dt.float32
FP16 = mybir.dt.float16
FP8 = mybir.dt.float8e4

CXW = JT + JT * B + HT * K   # fp16 pack: ct | xt | w2t  (560 cols)
XOFF = JT
WOFF = JT + JT * B

_CACHE = {}
LAST_RESULTS = None  # test.py reads this for profiling info


def _build_module():
    nc = bacc.Bacc("TRN2", target_bir_lowering=False, debug=False,
                   num_devices=NCORES)

    wt_d = nc.dram_tensor("wt", (K, 128, JT * OC), FP16, kind="ExternalInput")
    cxw_d = nc.dram_tensor("cxw", (128, CXW), FP16, kind="ExternalInput")
    w18_d = nc.dram_tensor("w18", (128, JT * H), FP8, kind="ExternalInput")
    b1r_d = nc.dram_tensor("b1r", (1, H), FP16, kind="ExternalInput")
    b2r_d = nc.dram_tensor("b2r", (1, K), FP16, kind="ExternalInput")
    kb_d = nc.dram_tensor("kb", (K, OC), FP16, kind="ExternalInput")
    y_d = nc.dram_tensor("y", (B, OC), F32, kind="ExternalOutput")
    # warmup sink: consumed so bacc's DCE keeps the PE warm-up matmuls
    ysink_d = nc.dram_tensor("ysink", (1, 1), F32, kind="ExternalOutput")

    n_warm1 = int(os.environ.get("KERNEL_WARMUP1", "12"))
    n_tailwarm = int(os.environ.get("KERNEL_TAILWARM", "36"))

    with tile.TileContext(nc) as tc:
        with (
            tc.tile_pool(name="cpool", bufs=1) as cpool,
            tc.tile_pool(name="wpool", bufs=1) as wpool,
            tc.tile_pool(name="ppool", bufs=1, space="PSUM") as ppool,
        ):
            # --- DMA: ONE bulk queue (sync). A measured lesson: running
            # both HWDGE queues concurrently drops combined throughput to
            # ~337 GB/s vs ~410 single-queue (per-packet round-robin
            # interference), so everything big goes on sync, ordered by
            # need: x/condition pack first (stationaries), then slabs,
            # with the alpha-MLP weights mid-stream (the per-k PSUM design
            # only needs `e` before the final combine). Slab 7 lands in 4
            # column quarters for a tight tail. ---
            slabs = [wpool.tile((128, JT * OC), FP16, tag="wt_slab", bufs=K,
                                name=f"wt_slab{k}")
                     for k in range(K)]
            cxw_sb = cpool.tile((128, CXW), FP16)
            w18_sb = cpool.tile((128, JT * H), FP8)
            kb_sb = cpool.tile((K, OC), FP16)
            b1r_sb = cpool.tile((1, H), FP16)
            b2r_sb = cpool.tile((1, K), FP16)

            # w1-fp8 chunks ride between slabs so the queue always has >=2
            # bulk transfers pending (a cluster of small DMAs serializes on
            # the framework's DMA-semaphore-reuse waits and drains the
            # stream). The 3 tiny bias loads go on the otherwise-idle
            # scalar ring for the same reason.
            nc.sync.dma_start(cxw_sb[:], cxw_d.ap())
            nc.sync.dma_start(w18_sb[:], w18_d.ap())
            for k in range(7):
                nc.sync.dma_start(slabs[k][:], wt_d.ap()[k])
            QC = JT * OC // 4  # 1024 cols per quarter
            for q in range(4):
                nc.sync.dma_start(slabs[7][:, q * QC:(q + 1) * QC],
                                  wt_d.ap()[7][:, q * QC:(q + 1) * QC])
            nc.scalar.dma_start(kb_sb[:], kb_d.ap())
            nc.scalar.dma_start(b1r_sb[:], b1r_d.ap())
            nc.scalar.dma_start(b2r_sb[:], b2r_d.ap())

            # --- constants ---
            one1h = cpool.tile((1, 1), FP16)
            nc.gpsimd.memset(one1h[:], 1.0)
            ones_b = cpool.tile((1, B), FP16)
            nc.gpsimd.memset(ones_b[:], 1.0)

            # --- PE warm-up: dependency-free matmuls ramp the HAM clock ---
            dum_a = cpool.tile((128, B), FP16)
            nc.gpsimd.memset(dum_a[:], 0.0)
            dum_b = cpool.tile((128, OC), FP16)
            nc.gpsimd.memset(dum_b[:], 0.0)
            dum_psum = ppool.tile((B, OC), F32)
            dum_sink = cpool.tile((1, 1), F32)

            def warmup(n):
                for _ in range(n):
                    nc.tensor.matmul(dum_psum[:], dum_a[:], dum_b[:],
                                     start=True, stop=True)

            warmup(n_warm1)

            # --- main contraction: raw x @ W_k^T per k. PSUM deps are
            # tile-granular and banks are 2 KB, so k=0..5 pair up into
            # three (B, 2*OC) tiles (their combines are e-gated anyway)
            # while k=6,7 get their own tiles to keep the tail short. PE
            # program order interleaves slab groups with the MLP so
            # whichever data lands first keeps the PE busy. ---
            mp01 = ppool.tile((B, 2 * OC), F32)
            mp23 = ppool.tile((B, 2 * OC), F32)
            mp45 = ppool.tile((B, 2 * OC), F32)
            mp6 = ppool.tile((B, OC), F32)
            mp7 = ppool.tile((B, OC), F32)

            def mp_ap(k):
                if k >= 6:
                    return (mp6 if k == 6 else mp7)[:]
                return (mp01, mp23, mp45)[k // 2][:, (k % 2) * OC:
                                                  (k % 2 + 1) * OC]

            def slab_group(k):
                for j in range(JT):
                    nc.tensor.matmul(
                        mp_ap(k),
                        cxw_sb[:, XOFF + j * B:XOFF + (j + 1) * B],
                        slabs[k][:, j * OC:(j + 1) * OC],
                        start=(j == 0), stop=(j == JT - 1),
                    )

            # --- alpha MLP: h = relu(cond @ w1 + b1), fp8 weights ---
            psum_h = ppool.tile((1, H), F32, tag="mlp")
            for t in range(JT):
                nc.tensor.matmul(
                    psum_h[:],
                    cxw_sb[:, t:t + 1],                  # ct column t
                    w18_sb[:, t * H:(t + 1) * H],        # w1 fp8 slab t
                    start=(t == 0), stop=False,
                )
            nc.tensor.matmul(psum_h[:], one1h[:], b1r_sb[:],
                             start=False, stop=True)
            h_sb = cpool.tile((1, H), FP16)
            nc.scalar.activation(h_sb[:], psum_h[:],
                                 mybir.ActivationFunctionType.Relu)

            # transpose h (1,512) -> hT (128,4) via tiny matmuls vs ones
            psum_ht = ppool.tile((128, HT), F32, tag="mlp")
            for q in range(HT):
                nc.tensor.matmul(
                    psum_ht[:, q:q + 1],
                    h_sb[:, q * 128:(q + 1) * 128],
                    one1h[:],
                    start=True, stop=True,
                )
            ht_sb = cpool.tile((128, HT), FP16)
            nc.vector.tensor_copy(ht_sb[:], psum_ht[:])

            # scores row (1, 8) = sum_q hT[:,q].T @ w2t[:,q,:] + b2
            psum_s = ppool.tile((1, K), F32, tag="mlp")
            for q in range(HT):
                nc.tensor.matmul(
                    psum_s[:],
                    ht_sb[:, q:q + 1],
                    cxw_sb[:, WOFF + q * K:WOFF + (q + 1) * K],
                    start=(q == 0), stop=False,
                )
            nc.tensor.matmul(psum_s[:], one1h[:], b2r_sb[:],
                             start=False, stop=True)

            # softmax, normalized up front: en = exp(s) / sum(exp(s));
            # no max-subtraction (scores are O(1) for this model family)
            e_sb = cpool.tile((1, K), F32)
            nc.scalar.activation(e_sb[:], psum_s[:],
                                 mybir.ActivationFunctionType.Exp)
            esum = cpool.tile((1, 1), F32)
            nc.vector.reduce_sum(esum[:], e_sb[:], axis=mybir.AxisListType.X)
            rinv = cpool.tile((1, 1), F32)
            nc.vector.reciprocal(rinv[:], esum[:])
            en16 = cpool.tile((1, K), FP16)
            nc.vector.tensor_scalar_mul(en16[:], e_sb[:], rinv[:])

            # broadcast en to all B partitions (for the combine scalars)
            psum_e = ppool.tile((B, K), F32, tag="mlp")
            nc.tensor.matmul(psum_e[:], ones_b[:], en16[:],
                             start=True, stop=True)
            e32_sb = cpool.tile((B, K), F32)
            nc.vector.tensor_copy(e32_sb[:], psum_e[:])

            # en column (8,1) -> e-weighted bias row (1, OC) -> broadcast
            # to (B, OC): the combine chain's initial accumulator
            psum_ac = ppool.tile((K, 1), F32, tag="mlp")
            nc.tensor.matmul(psum_ac[:], en16[:], one1h[:],
                             start=True, stop=True)
            e_c = cpool.tile((K, 1), FP16)
            nc.vector.tensor_copy(e_c[:], psum_ac[:])
            psum_bb = ppool.tile((1, OC), F32, tag="mlp")
            nc.tensor.matmul(psum_bb[:], e_c[:], kb_sb[:],
                             start=True, stop=True)
            aggb_sb = cpool.tile((1, OC), FP16)
            nc.vector.tensor_copy(aggb_sb[:], psum_bb[:])
            psum_cb = ppool.tile((B, OC), F32, tag="mlp")
            nc.tensor.matmul(psum_cb[:], ones_b[:], aggb_sb[:],
                             start=True, stop=True)

            for k in range(K):
                slab_group(k)

            # --- tail warm: keep the PE sequencer at speed through the
            # NEFF epilogue's distributed semaphore clear ---
            warmup(n_tailwarm)
            nc.vector.tensor_copy(dum_sink[:], dum_psum[0:1, 0:1])
            nc.scalar.dma_start(ysink_d.ap(), dum_sink[:])

            # --- combine: acc' = psum_k * en_k + acc, one fused DVE op
            # per k; k<7 complete during the stream, only k=7 is tail.
            # The last combine splits into OC halves so each output half
            # DMAs (on its own queue) as soon as its half is done. ---
            acc_a = cpool.tile((B, OC), F32)
            acc_b = cpool.tile((B, OC), F32)
            y_sb = cpool.tile((B, OC), F32)
            nc.vector.tensor_copy(acc_a[:], psum_cb[:])
            cur, nxt = acc_a, acc_b
            for k in range(K - 1):
                nc.vector.scalar_tensor_tensor(
                    nxt[:],
                    mp_ap(k),
                    e32_sb[:, k:k + 1],
                    cur[:],
                    op0=mybir.AluOpType.mult,
                    op1=mybir.AluOpType.add,
                )
                cur, nxt = nxt, cur
            HOC = OC // 2
            for h, eng in ((0, nc.scalar), (1, nc.sync)):
                sl = slice(h * HOC, (h + 1) * HOC)
                nc.vector.scalar_tensor_tensor(
                    y_sb[:, sl],
                    mp7[:, sl],
                    e32_sb[:, K - 1:K],
                    cur[:, sl],
                    op0=mybir.AluOpType.mult,
                    op1=mybir.AluOpType.add,
                )
                eng.dma_start(y_d.ap()[:, sl], y_sb[:, sl])

    nc.compile()
    return nc


def _prep_inputs(x, condition, w1, b1, w2, b2, kernels_weights, kernels_bias):
    """Layout-only host prep: slice per-core shards and retile for DMA."""
    import ml_dtypes
    f = np.float32
    f16 = np.float16
    f8 = ml_dtypes.float8_e4m3
    x = np.asarray(x, f)
    condition = np.asarray(condition, f)
    w1 = np.asarray(w1, f)
    b1 = np.asarray(b1, f)
    w2 = np.asarray(w2, f)
    b2 = np.asarray(b2, f)
    kernels_weights = np.asarray(kernels_weights, f)
    kernels_bias = np.asarray(kernels_bias, f)

    # xT tiled: xt[p, j*B + b] = x[b, j*128 + p]
    xt = np.ascontiguousarray(
        x.T.reshape(JT, 128, B).transpose(1, 0, 2)).reshape(128, JT * B)
    # w2 tiled as rhs: w2t[p, q*K + k] = w2[q*128 + p, k]
    w2t = np.ascontiguousarray(
        w2.reshape(HT, 128, K).transpose(1, 0, 2)).reshape(128, HT * K)
    ct = np.ascontiguousarray(condition.reshape(JT, 128).T)  # (128, JT)
    cxw = np.concatenate([ct, xt, w2t], axis=1).astype(f16)
    cxw = np.ascontiguousarray(cxw)

    # w1 tiled fp8: w18[p, t*H + h] = w1[t*128 + p, h]
    w18 = np.ascontiguousarray(
        w1.reshape(JT, 128, H).transpose(1, 0, 2)).reshape(128, JT * H)
    w18 = w18.astype(f8)

    b1r = np.ascontiguousarray(b1.reshape(1, H)).astype(f16)
    b2r = np.ascontiguousarray(b2.reshape(1, K)).astype(f16)

    in_maps = []
    for c in range(NCORES):
        osl = slice(c * OC, (c + 1) * OC)
        # W shard [k, o, i] -> tiles [k, p, j, o] with i = j*128 + p
        wt = np.ascontiguousarray(
            kernels_weights[:, osl, :].reshape(K, OC, JT, 128)
            .transpose(0, 3, 2, 1)).reshape(K, 128, JT * OC).astype(f16)
        kb = np.ascontiguousarray(kernels_bias[:, osl]).astype(f16)
        in_maps.append({
            "wt": wt, "cxw": cxw, "w18": w18,
            "b1r": b1r, "b2r": b2r, "kb": kb,
        })
    return in_maps


def kernel(x, condition, w1, b1, w2, b2, kernels_weights, kernels_bias):
    global LAST_RESULTS
    if "nc" not in _CACHE:
        _CACHE["nc"] = _build_module()
    nc = _CACHE["nc"]

    in_maps = _prep_inputs(x, condition, w1, b1, w2, b2,
                           kernels_weights, kernels_bias)

    res = run_bass_kernel_spmd(nc, in_maps, core_ids=list(range(NCORES)))
    LAST_RESULTS = res

    out = np.concatenate([res.results[c]["y"] for c in range(NCORES)], axis=1)
    return np.ascontiguousarray(out, dtype=np.float32)


if __name__ == "__main__":
    rng = np.random.default_rng(0)
    ins = {
        "x": rng.standard_normal((B, IN), dtype=np.float32),
        "condition": rng.standard_normal((1, IN), dtype=np.float32),
        "w1": rng.standard_normal((IN, H), dtype=np.float32) * 0.02,
        "b1": np.zeros(H, np.float32),
        "w2": rng.standard_normal((H, K), dtype=np.float32) * 0.02,
        "b2": np.zeros(K, np.float32),
        "kernels_weights": rng.standard_normal((K, OUT, IN),
                                               dtype=np.float32) * 0.01,
        "kernels_bias": np.zeros((K, OUT), np.float32),
    }
    y = kernel(**ins)
    print("out", y.shape, y.dtype, float(np.abs(y).mean()))


# revision 14
# speedup vs baseline: 1.4233x; 1.0034x over previous
"""Trainium2 Bass kernel for DynamicCondLinear (MoE-routing style).

Math: condition batch is 1, so the softmax routing weights (K=8) are shared by
all 32 samples; out = sum_k a_k * (x @ W_k^T) + sum_k a_k * b_k with
a = softmax(relu(cond @ w1 + b1) @ w2 + b2).

Sharding: tensor-parallel over OUT channels (2048 / 8 cores = 256 per core).
Each core streams its 8 MiB fp16 weight shard from HBM once; that stream is
the roofline (~20 us at the 412 GB/s measured 8KiB-packet rate).

v2 schedule (trace-driven redesign of the 43.3 us baseline):
 - per-k PSUM groups: slab k's matmuls accumulate raw x @ W_k^T into a
   dedicated (B, OC) PSUM slice, so the main contraction starts as soon as
   x + slab 0 land (~4 us) and never waits on the alpha MLP. The e-weighted
   combine runs as one fused DVE op per k (scalar_tensor_tensor:
   acc' = psum_k * e_k + acc), staggered with slab completions; only the
   k=7 op is on the tail.
 - single bulk HWDGE queue: concurrent queues measurably interfere (~337
   GB/s combined vs ~410 alone), so all streaming rides the sync ring with
   >=2 KiB descriptors; the scalar ring only carries one output half at the
   end. Last slab lands in 4 column quarters to keep the PE trailing the
   final bytes by <0.5 us.
 - w1 ships as fp8e4m3 (1 MiB instead of 2 MiB bf16): measured end-to-end
   rel err 6.9e-3 vs the 2e-2 gate. The PE upconverts fp8 rhs against the
   fp16 condition stationary.
 - softmax normalization is folded into the routing weights (e_k / sum(e))
   before the combine, so the output needs no final rescale pass.
 - dependency-free warm-up matmuls hold the PE HAM clock up through the DMA
   prefix; a tail batch keeps the sequencer hot into the NEFF epilogue's
   distributed semaphore clear (which otherwise runs at the LOW p-state).

Host-side prep is layout-only (transpose/reshape/cast for DMA-friendly
tiling); all math happens on-device.
"""

import os
import sys

import numpy as np

if "/opt/trn_rl_repo" not in sys.path:
    sys.path.insert(0, "/opt/trn_rl_repo")

import concourse.bacc as bacc
import concourse.mybir as mybir
import concourse.tile as tile
from concourse.bass_utils import run_bass_kernel_spmd

B, IN, OUT, K, H = 32, 2048, 2048, 8, 512
NCORES = 8
OC = OUT // NCORES  # 256 out channels per core
JT = IN // 128      # 16 contraction tiles
HT = H // 128       # 4 hidden tiles

F32 = mybir.dt.float32
FP16 = mybir.dt.float16
FP8 = mybir.dt.float8e4

CXW = JT + JT * B + HT * K   # fp16 pack: ct | xt | w2t  (560 cols)
XOFF = JT
WOFF = JT + JT * B

_CACHE = {}
LAST_RESULTS = None  # test.py reads this for profiling info


def _build_module():
    nc = bacc.Bacc("TRN2", target_bir_lowering=False, debug=False,
                   num_devices=NCORES)

    wtp_d = nc.dram_tensor("wtp", (3, 128, 2 * JT * OC), FP16,
                           kind="ExternalInput")
    wt67_d = nc.dram_tensor("wt67", (2, 128, JT * OC), FP16,
                            kind="ExternalInput")
    cxw_d = nc.dram_tensor("cxw", (128, CXW), FP16, kind="ExternalInput")
    w18_d = nc.dram_tensor("w18", (128, JT * H), FP8, kind="ExternalInput")
    b1r_d = nc.dram_tensor("b1r", (1, H), FP16, kind="ExternalInput")
    b2r_d = nc.dram_tensor("b2r", (1, K), FP16, kind="ExternalInput")
    kb_d = nc.dram_tensor("kb", (K, OC), FP16, kind="ExternalInput")
    y_d = nc.dram_tensor("y", (B, OC), F32, kind="ExternalOutput")
    # warmup sink: consumed so bacc's DCE keeps the PE warm-up matmuls
    ysink_d = nc.dram_tensor("ysink", (1, 1), F32, kind="ExternalOutput")

    n_warm1 = int(os.environ.get("KERNEL_WARMUP1", "12"))
    n_tailwarm = int(os.environ.get("KERNEL_TAILWARM", "0"))

    with tile.TileContext(nc) as tc:
        with (
            tc.tile_pool(name="cpool", bufs=1) as cpool,
            tc.tile_pool(name="wpool", bufs=1) as wpool,
            tc.tile_pool(name="ppool", bufs=1, space="PSUM") as ppool,
        ):
            # --- DMA: ONE bulk queue (sync). A measured lesson: running
            # both HWDGE queues concurrently drops combined throughput to
            # ~337 GB/s vs ~410 single-queue (per-packet round-robin
            # interference), so everything big goes on sync, ordered by
            # need: x/condition pack first (stationaries), then slabs,
            # with the alpha-MLP weights mid-stream (the per-k PSUM design
            # only needs `e` before the final combine). Slab 7 lands in 4
            # column quarters for a tight tail. ---
            pairs = [wpool.tile((128, 2 * JT * OC), FP16, tag="wt_pair",
                                bufs=3, name=f"wt_pair{p}")
                     for p in range(3)]
            st6 = wpool.tile((128, JT * OC), FP16)
            st7 = wpool.tile((128, JT * OC), FP16)
            cxw_sb = cpool.tile((128, CXW), FP16)
            w18_sb = cpool.tile((128, JT * H), FP8)
            kb_sb = cpool.tile((K, OC), FP16)
            b1r_sb = cpool.tile((1, H), FP16)
            b2r_sb = cpool.tile((1, K), FP16)

            # w1-fp8 chunks ride between slabs so the queue always has >=2
            # bulk transfers pending (a cluster of small DMAs serializes on
            # the framework's DMA-semaphore-reuse waits and drains the
            # stream). The 3 tiny bias loads go on the otherwise-idle
            # scalar ring for the same reason.
            nc.sync.dma_start(cxw_sb[:], cxw_d.ap())
            nc.sync.dma_start(w18_sb[:], w18_d.ap())
            for p in range(3):
                nc.sync.dma_start(pairs[p][:], wtp_d.ap()[p])
            nc.sync.dma_start(st6[:], wt67_d.ap()[0])
            QC = JT * OC // 4  # 1024 cols per quarter
            for q in range(4):
                nc.sync.dma_start(st7[:, q * QC:(q + 1) * QC],
                                  wt67_d.ap()[1][:, q * QC:(q + 1) * QC])
            nc.scalar.dma_start(kb_sb[:], kb_d.ap())
            nc.scalar.dma_start(b1r_sb[:], b1r_d.ap())
            nc.scalar.dma_start(b2r_sb[:], b2r_d.ap())

            # --- constants ---
            one1h = cpool.tile((1, 1), FP16)
            nc.gpsimd.memset(one1h[:], 1.0)
            ones_b = cpool.tile((1, B), FP16)
            nc.gpsimd.memset(ones_b[:], 1.0)

            # --- PE warm-up: dependency-free matmuls ramp the HAM clock ---
            dum_a = cpool.tile((128, B), FP16)
            nc.gpsimd.memset(dum_a[:], 0.0)
            dum_b = cpool.tile((128, OC), FP16)
            nc.gpsimd.memset(dum_b[:], 0.0)
            dum_psum = ppool.tile((B, OC), F32)
            dum_sink = cpool.tile((1, 1), F32)

            def warmup(n):
                for _ in range(n):
                    nc.tensor.matmul(dum_psum[:], dum_a[:], dum_b[:],
                                     start=True, stop=True)

            warmup(n_warm1)

            # --- main contraction: raw x @ W_k^T per k. PSUM deps are
            # tile-granular and banks are 2 KB, so k=0..5 pair up into
            # three (B, 2*OC) tiles (their combines are e-gated anyway)
            # while k=6,7 get their own tiles to keep the tail short. PE
            # program order interleaves slab groups with the MLP so
            # whichever data lands first keeps the PE busy. ---
            mp01 = ppool.tile((B, 2 * OC), F32)
            mp23 = ppool.tile((B, 2 * OC), F32)
            mp45 = ppool.tile((B, 2 * OC), F32)
            mp6 = ppool.tile((B, OC), F32)
            mp7 = ppool.tile((B, OC), F32)

            def mp_ap(k):
                if k >= 6:
                    return (mp6 if k == 6 else mp7)[:]
                return (mp01, mp23, mp45)[k // 2][:, (k % 2) * OC:
                                                  (k % 2 + 1) * OC]

            def slab_rhs(k, j):
                if k >= 6:
                    t = st6 if k == 6 else st7
                    return t[:, j * OC:(j + 1) * OC]
                base = (k % 2) * JT * OC
                return pairs[k // 2][:, base + j * OC:base + (j + 1) * OC]

            def slab_group(k):
                for j in range(JT):
                    nc.tensor.matmul(
                        mp_ap(k),
                        cxw_sb[:, XOFF + j * B:XOFF + (j + 1) * B],
                        slab_rhs(k, j),
                        start=(j == 0), stop=(j == JT - 1),
                    )

            # --- alpha MLP: h = relu(cond @ w1 + b1), fp8 weights ---
            psum_h = ppool.tile((1, H), F32, tag="mlp")
            for t in range(JT):
                nc.tensor.matmul(
                    psum_h[:],
                    cxw_sb[:, t:t + 1],                  # ct column t
                    w18_sb[:, t * H:(t + 1) * H],        # w1 fp8 slab t
                    start=(t == 0), stop=False,
                )
            nc.tensor.matmul(psum_h[:], one1h[:], b1r_sb[:],
                             start=False, stop=True)
            h_sb = cpool.tile((1, H), FP16)
            nc.scalar.activation(h_sb[:], psum_h[:],
                                 mybir.ActivationFunctionType.Relu)

            # transpose h (1,512) -> hT (128,4) via tiny matmuls vs ones
            psum_ht = ppool.tile((128, HT), F32, tag="mlp")
            for q in range(HT):
                nc.tensor.matmul(
                    psum_ht[:, q:q + 1],
                    h_sb[:, q * 128:(q + 1) * 128],
                    one1h[:],
                    start=True, stop=True,
                )
            ht_sb = cpool.tile((128, HT), FP16)
            nc.vector.tensor_copy(ht_sb[:], psum_ht[:])

            # scores row (1, 8) = sum_q hT[:,q].T @ w2t[:,q,:] + b2
            psum_s = ppool.tile((1, K), F32, tag="mlp")
            for q in range(HT):
                nc.tensor.matmul(
                    psum_s[:],
                    ht_sb[:, q:q + 1],
                    cxw_sb[:, WOFF + q * K:WOFF + (q + 1) * K],
                    start=(q == 0), stop=False,
                )
            nc.tensor.matmul(psum_s[:], one1h[:], b2r_sb[:],
                             start=False, stop=True)

            # softmax, normalized up front: en = exp(s) / sum(exp(s));
            # no max-subtraction (scores are O(1) for this model family)
            e_sb = cpool.tile((1, K), F32)
            nc.scalar.activation(e_sb[:], psum_s[:],
                                 mybir.ActivationFunctionType.Exp)
            esum = cpool.tile((1, 1), F32)
            nc.vector.reduce_sum(esum[:], e_sb[:], axis=mybir.AxisListType.X)
            rinv = cpool.tile((1, 1), F32)
            nc.vector.reciprocal(rinv[:], esum[:])
            en16 = cpool.tile((1, K), FP16)
            nc.vector.tensor_scalar_mul(en16[:], e_sb[:], rinv[:])

            # broadcast en to all B partitions (for the combine scalars)
            psum_e = ppool.tile((B, K), F32, tag="mlp")
            nc.tensor.matmul(psum_e[:], ones_b[:], en16[:],
                             start=True, stop=True)
            e32_sb = cpool.tile((B, K), F32)
            nc.vector.tensor_copy(e32_sb[:], psum_e[:])

            # en column (8,1) -> e-weighted bias row (1, OC) -> broadcast
            # to (B, OC): the combine chain's initial accumulator
            psum_ac = ppool.tile((K, 1), F32, tag="mlp")
            nc.tensor.matmul(psum_ac[:], en16[:], one1h[:],
                             start=True, stop=True)
            e_c = cpool.tile((K, 1), FP16)
            nc.vector.tensor_copy(e_c[:], psum_ac[:])
            psum_bb = ppool.tile((1, OC), F32, tag="mlp")
            nc.tensor.matmul(psum_bb[:], e_c[:], kb_sb[:],
                             start=True, stop=True)
            aggb_sb = cpool.tile((1, OC), FP16)
            nc.vector.tensor_copy(aggb_sb[:], psum_bb[:])
            psum_cb = ppool.tile((B, OC), F32, tag="mlp")
            nc.tensor.matmul(psum_cb[:], ones_b[:], aggb_sb[:],
                             start=True, stop=True)

            for k in range(K):
                slab_group(k)

            # --- tail warm: keep the PE sequencer at speed through the
            # NEFF epilogue's distributed semaphore clear ---
            warmup(n_tailwarm)
            nc.vector.tensor_copy(dum_sink[:], dum_psum[0:1, 0:1])
            nc.scalar.dma_start(ysink_d.ap(), dum_sink[:])

            # --- combine: acc' = psum_k * en_k + acc, one fused DVE op
            # per k; k<7 complete during the stream, only k=7 is tail.
            # The last combine splits into OC halves so each output half
            # DMAs (on its own queue) as soon as its half is done. ---
            acc_a = cpool.tile((B, OC), F32)
            acc_b = cpool.tile((B, OC), F32)
            y_sb = cpool.tile((B, OC), F32)
            nc.vector.tensor_copy(acc_a[:], psum_cb[:])
            cur, nxt = acc_a, acc_b
            for k in range(K - 1):
                nc.vector.scalar_tensor_tensor(
                    nxt[:],
                    mp_ap(k),
                    e32_sb[:, k:k + 1],
                    cur[:],
                    op0=mybir.AluOpType.mult,
                    op1=mybir.AluOpType.add,
                )
                cur, nxt = nxt, cur
            HOC = OC // 2
            for h, eng in ((0, nc.scalar), (1, nc.sync)):
                sl = slice(h * HOC, (h + 1) * HOC)
                nc.vector.scalar_tensor_tensor(
                    y_sb[:, sl],
                    mp7[:, sl],
                    e32_sb[:, K - 1:K],
                    cur[:, sl],
                    op0=mybir.AluOpType.mult,
                    op1=mybir.AluOpType.add,
                )
                eng.dma_start(y_d.ap()[:, sl], y_sb[:, sl])

    nc.compile()
    return nc


def _prep_inputs(x, condition, w1, b1, w2, b2, kernels_weights, kernels_bias):
    """Layout-only host prep: slice per-core shards and retile for DMA."""
    import ml_dtypes
    f = np.float32
    f16 = np.float16
    f8 = ml_dtypes.float8_e4m3
    x = np.asarray(x, f)
    condition = np.asarray(condition, f)
    w1 = np.asarray(w1, f)
    b1 = np.asarray(b1, f)
    w2 = np.asarray(w2, f)
    b2 = np.asarray(b2, f)
    kernels_weights = np.asarray(kernels_weights, f)
    kernels_bias = np.asarray(kernels_bias, f)

    # xT tiled: xt[p, j*B + b] = x[b, j*128 + p]
    xt = np.ascontiguousarray(
        x.T.reshape(JT, 128, B).transpose(1, 0, 2)).reshape(128, JT * B)
    # w2 tiled as rhs: w2t[p, q*K + k] = w2[q*128 + p, k]
    w2t = np.ascontiguousarray(
        w2.reshape(HT, 128, K).transpose(1, 0, 2)).reshape(128, HT * K)
    ct = np.ascontiguousarray(condition.reshape(JT, 128).T)  # (128, JT)
    cxw = np.concatenate([ct, xt, w2t], axis=1).astype(f16)
    cxw = np.ascontiguousarray(cxw)

    # w1 tiled fp8: w18[p, t*H + h] = w1[t*128 + p, h]
    w18 = np.ascontiguousarray(
        w1.reshape(JT, 128, H).transpose(1, 0, 2)).reshape(128, JT * H)
    w18 = w18.astype(f8)

    b1r = np.ascontiguousarray(b1.reshape(1, H)).astype(f16)
    b2r = np.ascontiguousarray(b2.reshape(1, K)).astype(f16)

    in_maps = []
    for c in range(NCORES):
        osl = slice(c * OC, (c + 1) * OC)
        # W shard [k, o, i] -> tiles [k, p, j, o] with i = j*128 + p;
        # slabs 0-5 pair up so each DMA line is 16 KiB contiguous
        wt = np.ascontiguousarray(
            kernels_weights[:, osl, :].reshape(K, OC, JT, 128)
            .transpose(0, 3, 2, 1)).reshape(K, 128, JT * OC).astype(f16)
        wtp = np.ascontiguousarray(
            wt[:6].reshape(3, 2, 128, JT * OC).transpose(0, 2, 1, 3)
            .reshape(3, 128, 2 * JT * OC))
        wt67 = np.ascontiguousarray(wt[6:])
        kb = np.ascontiguousarray(kernels_bias[:, osl]).astype(f16)
        in_maps.append({
            "wtp": wtp, "wt67": wt67, "cxw": cxw, "w18": w18,
            "b1r": b1r, "b2r": b2r, "kb": kb,
        })
    return in_maps


def kernel(x, condition, w1, b1, w2, b2, kernels_weights, kernels_bias):
    global LAST_RESULTS
    if "nc" not in _CACHE:
        _CACHE["nc"] = _build_module()
    nc = _CACHE["nc"]

    in_maps = _prep_inputs(x, condition, w1, b1, w2, b2,
                           kernels_weights, kernels_bias)

    res = run_bass_kernel_spmd(nc, in_maps, core_ids=list(range(NCORES)))
    LAST_RESULTS = res

    out = np.concatenate([res.results[c]["y"] for c in range(NCORES)], axis=1)
    return np.ascontiguousarray(out, dtype=np.float32)


if __name__ == "__main__":
    rng = np.random.default_rng(0)
    ins = {
        "x": rng.standard_normal((B, IN), dtype=np.float32),
        "condition": rng.standard_normal((1, IN), dtype=np.float32),
        "w1": rng.standard_normal((IN, H), dtype=np.float32) * 0.02,
        "b1": np.zeros(H, np.float32),
        "w2": rng.standard_normal((H, K), dtype=np.float32) * 0.02,
        "b2": np.zeros(K, np.float32),
        "kernels_weights": rng.standard_normal((K, OUT, IN),
                                               dtype=np.float32) * 0.01,
        "kernels_bias": np.zeros((K, OUT), np.float32),
    }
    y = kernel(**ins)
    print("out", y.shape, y.dtype, float(np.abs(y).mean()))
